# revision 5
# baseline (speedup 1.0000x reference)
"""Distributed attention-head kernel for 8 TRN2 NeuronCores.

Problem: B=4, S=4096, D=1024, H=64
  qs = LN(xs @ Wq); ks = LN(xs @ Wk); vs = xs @ Wv
  out = softmax(qs ks^T / 8) vs          (per batch, full attention)

Sharding: 2 cores per batch element; each core computes the full K/V of its
batch (redundantly, cheap) and attention for its own half of the queries
(2048 rows). No collectives.

v2 design notes (HW-measured on top of the v1 kernel):
  * The kernel is ACT(exp)-bound: 64 exp tiles of [128,1024] ~= 67us.
    Everything else is arranged to hide under that stream.
  * Scores are computed in row-tiled PAIRS: k-tile pairs (8m+i, 8m+4+i)
    occupy PE row groups 0:63 / 64:127 concurrently (K=64 each), so a
    512-column feed produces TWO k-tiles of scores^T.  kt2 holds the pair
    layout [128, 16*128]; qt2 replicates Q^T into both partition halves.
  * PV is row-tiled the same way with zero layout change: each 128-key
    tile's V' splits naturally at partition 64; low/high key halves
    accumulate into separate psO_A/psO_B banks, summed at the end by DVE.
  * One unified 8-bank PSUM budget: a [128,1024] f32 "wide" pool (3 bufs,
    6 banks) serves projection psA|psB, stats+V-transposes, the rsig/mu
    broadcast pair, and the phase-2 score tiles; psO_A/psO_B take the
    last 2 banks.  This lets second-half projection weave into phase 2.
  * Program order interleaves: blocks 0,1 -> stats/norm -> blocks 2,3 ->
    stats/norm -> (qc0 scores | block4) -> (qc1 | block5) ... so the exp
    stream starts ~11us in and the PE fills its gaps with projection.
  * raws kept in bf16 (halves DVE copy cost); mu kept only in bf16; var
    computed in-place; LN stats ln/exp run per 2-block chunk to shorten
    the critical path (table thrash Ln<->Exp is ~2.6us per chunk).
"""

import numpy as np
import ml_dtypes

S = 4096
D = 1024
H = 64
HQ = 2048  # queries owned per core
NB = S // 512  # 8 s-blocks of 512
DT = D // 128  # 8 d-tiles
NKT = S // 128  # 32 k-tiles
NPAIR = NKT // 2  # 16 row-tiled score pairs
BF16 = ml_dtypes.bfloat16

_CACHE = {}


def _build_nc():
    import concourse.bacc as bacc
    import concourse.mybir as mybir
    import concourse.tile as tile

    f32 = mybir.dt.float32
    bf16 = mybir.dt.bfloat16
    EXP = mybir.ActivationFunctionType.Exp
    LN_ = mybir.ActivationFunctionType.Ln

    nc = bacc.Bacc("TRN2", target_bir_lowering=False, debug=False, num_devices=8)

    xst_d = nc.dram_tensor("xst", [D, S], bf16, kind="ExternalInput")
    wa_d = nc.dram_tensor("wa", [D, 128], bf16, kind="ExternalInput")
    wb_d = nc.dram_tensor("wb", [D, 66], bf16, kind="ExternalInput")
    sel_d = nc.dram_tensor("sel", [128, 2], bf16, kind="ExternalInput")
    selq_d = nc.dram_tensor("selq", [2, 64], bf16, kind="ExternalInput")
    selk_d = nc.dram_tensor("selk", [2, 64], bf16, kind="ExternalInput")
    selqm_d = nc.dram_tensor("selqm", [2, 64], bf16, kind="ExternalInput")
    selkm_d = nc.dram_tensor("selkm", [2, 64], bf16, kind="ExternalInput")
    ident_d = nc.dram_tensor("ident", [64, 64], f32, kind="ExternalInput")
    outT_d = nc.dram_tensor("outT", [65, HQ], f32, kind="ExternalOutput")

    with tile.TileContext(nc) as tc:
        with (
            tc.tile_pool(name="const", bufs=1) as cpool,
            tc.tile_pool(name="big", bufs=1) as big,
            tc.tile_pool(name="xs", bufs=3) as xpool,
            tc.tile_pool(name="sq", bufs=2) as sqpool,
            tc.tile_pool(name="wide", bufs=3, space="PSUM") as wpool,
            tc.tile_pool(name="psOA", bufs=1, space="PSUM") as psOA_pool,
            tc.tile_pool(name="psOB", bufs=1, space="PSUM") as psOB_pool,
            tc.tile_pool(name="ebuf", bufs=3) as epool,
            tc.tile_pool(name="ot", bufs=2) as otpool,
        ):
            # constants
            wa_sb = cpool.tile([128, DT, 128], bf16)
            wb_sb = cpool.tile([128, DT, 66], bf16)
            sel_sb = cpool.tile([128, 2], bf16)
            selq_sb = cpool.tile([2, 64], bf16)
            selk_sb = cpool.tile([2, 64], bf16)
            selqm_sb = cpool.tile([2, 64], bf16)
            selkm_sb = cpool.tile([2, 64], bf16)
            ident_sb = cpool.tile([64, 64], f32)
            zero_sb = cpool.tile([128, 1], f32)
            eps_sb = cpool.tile([16, 1], f32)
            nc.vector.memset(zero_sb[:], 0.0)
            nc.vector.memset(eps_sb[:], 1e-5)
            nc.gpsimd.dma_start(out=wa_sb[:], in_=wa_d.ap().rearrange("(t p) m -> p t m", p=128))
            nc.gpsimd.dma_start(out=wb_sb[:], in_=wb_d.ap().rearrange("(t p) m -> p t m", p=128))
            nc.gpsimd.dma_start(out=sel_sb[:], in_=sel_d[:])
            nc.gpsimd.dma_start(out=selq_sb[:], in_=selq_d[:])
            nc.gpsimd.dma_start(out=selk_sb[:], in_=selk_d[:])
            nc.gpsimd.dma_start(out=selqm_sb[:], in_=selqm_d[:])
            nc.gpsimd.dma_start(out=selkm_sb[:], in_=selkm_d[:])
            nc.gpsimd.dma_start(out=ident_sb[:], in_=ident_d[:])

            # big persistent buffers
            raws = big.tile([128, NB, 512], bf16)  # rows 0-63 Q^T_raw, 64-127 K^T_raw
            vt_sb = big.tile([64, S], f32)         # V^T staging for PE transposes
            qt2 = big.tile([128, HQ], bf16)        # normalized Q^T, both halves
            kt2 = big.tile([128, NPAIR * 128], bf16)  # pair layout: lo rows=tile 8m+i, hi=8m+4+i
            vp = big.tile([128, NKT, 65], bf16)    # V' = [V | ones]
            mu_bf = big.tile([2, S], bf16)         # row 0 = 8mu_q, row 1 = -8mu_k
            musq_sb = big.tile([2, S], f32)        # mu^2 then var (in place)
            lnv_sb = big.tile([2, S], f32)
            rsig_bf = big.tile([2, S], bf16)       # row 0 = rq, row 1 = rk
            partial = big.tile([65, 4, 512], f32)  # h0 partial PV sums per q-chunk

            nc.vector.memset(vp[:, :, 64:65], 1.0)

            xst_r = xst_d.ap().rearrange("(t p) s -> p t s", p=128)

            def proj_block(j):
                """DMA + project block j; stats matmul; V transposes."""
                blk = slice(j * 512, (j + 1) * 512)
                xst_j = xpool.tile([128, DT, 512], bf16, tag="xst")
                if j == 0:
                    # split the first block so the PE starts ASAP
                    nc.sync.dma_start(out=xst_j[:, 0:2, :], in_=xst_r[:, 0:2, blk])
                    nc.sync.dma_start(out=xst_j[:, 2:DT, :], in_=xst_r[:, 2:DT, blk])
                else:
                    nc.sync.dma_start(out=xst_j[:], in_=xst_r[:, :, blk])
                w1 = wpool.tile([128, 1024], f32, tag="wide")  # psA | psB
                for t in range(DT):
                    nc.tensor.matmul(w1[:, 0:512], wa_sb[:, t], xst_j[:, t, :],
                                     start=(t == 0), stop=(t == DT - 1))
                for t in range(DT):
                    nc.tensor.matmul(w1[0:66, 512:1024], wb_sb[:, t], xst_j[:, t, :],
                                     start=(t == 0), stop=(t == DT - 1))
                nc.vector.tensor_copy(raws[:, j, :], w1[:, 0:512])
                nc.vector.tensor_copy(vt_sb[:, blk], w1[0:64, 512:1024])
                nc.vector.tensor_copy(mu_bf[0:2, blk], w1[64:66, 512:1024])
                sq = sqpool.tile([128, 512], bf16, tag="sq")
                nc.gpsimd.tensor_mul(sq[:], raws[:, j, :], raws[:, j, :])
                w2 = wpool.tile([128, 1024], f32, tag="wide")  # psSt | psv x4
                nc.tensor.matmul(w2[0:2, 0:512], sel_sb[:], sq[:], start=True, stop=True)
                for i in range(4):
                    kti = j * 4 + i
                    nc.tensor.transpose(w2[:, 512 + i * 64: 512 + (i + 1) * 64],
                                        vt_sb[:, kti * 128:(kti + 1) * 128], ident_sb[:])
                    nc.vector.tensor_copy(vp[:, kti, 0:64],
                                          w2[:, 512 + i * 64: 512 + (i + 1) * 64])
                # mu^2, then (mu^2 - stats) in place; the Ln below negates via
                # its scale so the result is still ln(var/64 + eps)
                nc.vector.tensor_mul(musq_sb[:, blk], mu_bf[:, blk], mu_bf[:, blk])
                nc.vector.tensor_sub(musq_sb[:, blk], musq_sb[:, blk], w2[0:2, 0:512])

            def stats_chunk(b0, b1):
                """rsig for blocks [b0, b1): Ln then Exp on the packed var."""
                cs = slice(b0 * 512, b1 * 512)
                nc.scalar.activation(lnv_sb[:, cs], musq_sb[:, cs], LN_,
                                     bias=eps_sb[0:2], scale=-1.0 / 64.0)
                nc.scalar.activation(rsig_bf[:, cs], lnv_sb[:, cs], EXP,
                                     bias=zero_sb[0:2], scale=-0.5)

            def norm_block(j):
                """Normalize block j's K^T into kt2 (and Q^T into qt2 if own half)."""
                blk = slice(j * 512, (j + 1) * 512)
                m, even = j // 2, (j % 2 == 0)
                # kt2 destination: block 2m -> lo rows, block 2m+1 -> hi rows
                dst = kt2[0:64, m * 512:(m + 1) * 512] if even else \
                    kt2[64:128, m * 512:(m + 1) * 512]
                w3 = wpool.tile([128, 1024], f32, tag="wide")  # psRk|psMk
                nc.tensor.matmul(w3[0:64, 0:512], selk_sb[:], rsig_bf[0:2, blk],
                                 start=True, stop=True)
                nc.tensor.matmul(w3[64:128, 512:1024], selkm_sb[:], mu_bf[0:2, blk],
                                 start=True, stop=True)
                nc.vector.tensor_sub(dst, raws[64:128, j, :], w3[64:128, 512:1024])
                nc.vector.tensor_mul(dst, dst, w3[0:64, 0:512])
                if j < 4:
                    w4 = wpool.tile([128, 1024], f32, tag="wide")  # psRq|psMq
                    nc.tensor.matmul(w4[0:64, 0:512], selq_sb[:], rsig_bf[0:2, blk],
                                     start=True, stop=True)
                    nc.tensor.matmul(w4[64:128, 512:1024], selqm_sb[:], mu_bf[0:2, blk],
                                     start=True, stop=True)
                    nc.vector.tensor_sub(qt2[0:64, blk], raws[0:64, j, :],
                                         w4[64:128, 512:1024])
                    nc.vector.tensor_mul(qt2[0:64, blk], qt2[0:64, blk], w4[0:64, 0:512])
                    # replicate into the high partitions for row-tiled rhs
                    nc.gpsimd.dma_start(out=qt2[64:128, blk], in_=qt2[0:64, blk])

            def phase2_qc(qc, h, last=False):
                """Scores+exp+PV for q-chunk qc (512 q), half h (pairs 8h..8h+7)."""
                qs_ = slice(qc * 512, (qc + 1) * 512)
                psA = psOA_pool.tile([65, 512], f32, tag="oa")
                psB = psOB_pool.tile([65, 512], f32, tag="ob")
                for pi in range(8):
                    p = 8 * h + pi
                    mm = p // 4
                    klo = 8 * mm + (p % 4)
                    khi = klo + 4
                    psS = wpool.tile([128, 1024], f32, tag="wide")
                    nc.tensor.matmul(psS[:, 0:512],
                                     kt2[0:64, p * 128:(p + 1) * 128],
                                     qt2[0:64, qs_], start=True, stop=True)
                    nc.tensor.matmul(psS[:, 512:1024],
                                     kt2[64:128, p * 128:(p + 1) * 128],
                                     qt2[64:128, qs_], start=True, stop=True)
                    e = epool.tile([128, 1024], bf16, tag="e")
                    nc.scalar.activation(e[:], psS[:], EXP, bias=zero_sb[:], scale=0.125)
                    st = (pi == 0)
                    sp = (pi == 7)
                    nc.tensor.matmul(psA[:], vp[0:64, klo, :], e[0:64, 0:512],
                                     start=st, stop=False)
                    nc.tensor.matmul(psB[:], vp[64:128, klo, :], e[64:128, 0:512],
                                     start=st, stop=False)
                    nc.tensor.matmul(psA[:], vp[0:64, khi, :], e[0:64, 512:1024],
                                     start=False, stop=sp)
                    nc.tensor.matmul(psB[:], vp[64:128, khi, :], e[64:128, 512:1024],
                                     start=False, stop=sp)
                if not last:
                    # h0: stash A+B into the SBUF partial for this q-chunk
                    # (DVE may read only one PSUM operand per instruction)
                    nc.vector.tensor_copy(partial[:, qc, :], psA[:])
                    nc.vector.tensor_add(partial[:, qc, :], partial[:, qc, :], psB[:])
                else:
                    # h1: final = A + B + partial -> host (divide happens there)
                    ot = otpool.tile([65, 512], f32, tag="ot")
                    nc.vector.tensor_add(ot[:], partial[:, qc, :], psB[:])
                    nc.vector.tensor_add(ot[:], ot[:], psA[:])
                    nc.gpsimd.dma_start(out=outT_d[:, qc * 512:(qc + 1) * 512], in_=ot[:])

            # ---------------- schedule (program order = priority) -------------
            proj_block(0)
            proj_block(1)
            stats_chunk(0, 2)
            norm_block(0)
            norm_block(1)
            proj_block(2)
            proj_block(3)
            stats_chunk(2, 4)
            norm_block(2)
            norm_block(3)
            for qc in range(4):
                phase2_qc(qc, h=0, last=False)
                proj_block(4 + qc)
            stats_chunk(4, 8)
            for j in range(4, 8):
                norm_block(j)
            for qc in range(4):
                phase2_qc(qc, h=1, last=True)

    nc.finalize()
    return nc


def _get_nc():
    if "nc" not in _CACHE:
        _CACHE["nc"] = _build_nc()
    return _CACHE["nc"]


def _make_in_maps(xs_q, Wq, Wk, Wv):
    wa32 = np.concatenate([Wq, Wk], axis=1).astype(np.float32)
    wa = wa32.astype(BF16)
    # mu columns from the bf16-rounded weights so the folded identity is tight
    wab = wa.astype(np.float32)
    wmu_q = 8.0 * wab[:, :64].mean(axis=1, keepdims=True)
    wmu_k = -8.0 * wab[:, 64:].mean(axis=1, keepdims=True)
    wb = np.concatenate([Wv.astype(np.float32), wmu_q, wmu_k], axis=1).astype(BF16)
    sel = np.zeros((128, 2), BF16)
    sel[:64, 0] = 1
    sel[64:, 1] = 1
    # row-select lhsTs for the broadcast matmuls (columns = output rows)
    selq = np.zeros((2, 64), BF16); selq[0, :] = 1.0
    selk = np.zeros((2, 64), BF16); selk[1, :] = 1.0
    # mu pick rows, scaled so psM = mu: q row0 = 8mu_q -> +0.125; k row1 = -8mu_k -> -0.125
    selqm = np.zeros((2, 64), BF16); selqm[0, :] = 0.125
    selkm = np.zeros((2, 64), BF16); selkm[1, :] = -0.125
    ident = np.eye(64, dtype=np.float32)
    in_maps = []
    for c in range(8):
        b, h = c // 2, c % 2
        x = xs_q[b]
        q0 = h * HQ
        xr = np.concatenate([x[q0:q0 + HQ], x[:q0], x[q0 + HQ:]], axis=0)
        xst = np.ascontiguousarray(xr.T).astype(BF16)
        in_maps.append({
            "xst": xst, "wa": wa, "wb": wb, "sel": sel,
            "selq": selq, "selk": selk, "selqm": selqm, "selkm": selkm,
            "ident": ident,
        })
    return in_maps


def _ensure_ntff_hook():
    try:
        from antenv.axon_hooks import (
            get_axon_ntff_profile_hook, set_axon_ntff_profile_hook)
        if get_axon_ntff_profile_hook() is None:
            import sys as _sys
            if "/root/.axon_site/trn_agent_boot" not in _sys.path:
                _sys.path.insert(0, "/root/.axon_site/trn_agent_boot")
            import trn_boot
            h = trn_boot._ntff_profile_via_ctypes("/opt/axon/libaxon_pjrt.so")
            if h is not None:
                set_axon_ntff_profile_hook(h)
    except Exception:
        pass


def run(xs_q, Wq, Wk, Wv, trace=False):
    from concourse.bass_utils import run_bass_kernel_spmd
    if trace:
        _ensure_ntff_hook()
    nc = _get_nc()
    in_maps = _make_in_maps(xs_q, Wq, Wk, Wv)
    res = run_bass_kernel_spmd(nc, in_maps, list(range(8)), trace=trace)
    out = np.empty((4, S, H), np.float32)
    for c in range(8):
        b, h = c // 2, c % 2
        r = np.asarray(res.results[c]["outT"])
        out[b, h * HQ:(h + 1) * HQ] = (r[0:64] / r[64:65]).T
    return out, res


def kernel(xs_q, Wq, Wk, Wv):
    out, _ = run(xs_q, Wq, Wk, Wv, trace=False)
    return out
